# revision 28
# baseline (speedup 1.0000x reference)
"""GAT 2-layer network on 8 Trainium2 NeuronCores.

Strategy (edge-parallel, per the sharding hint "partition edges, replicate
node features"):
  - Nodes are sorted by in-degree and packed into 128-node tiles; tiles are
    dealt round-robin onto the 8 cores so every core runs the identical
    instruction stream (SPMD) over a shared per-step K schedule.
  - All FLOPs run on device across 3 launches:
      K1: xp1 = x @ W1 plus per-head attention dot products (s1, ad1).
      K2: per dst-tile segment softmax + message aggregation for layer 1,
          ELU, then xp2 = h @ W2ext (fused) -> layer-2 node table.
      K3: layer-2 segment softmax + aggregation + bias + log_softmax.
  - Between launches the host only does index-based data movement: it
    replicates the device-computed per-node tables into per-edge-slot
    streams (degree-padded, p-major) so each device step reads purely
    sequential DMA. No floating-point math happens on the host.
  - The edge streams and all bulk elementwise work run in fp16 (device
    casts on device; host only moves fp16 bytes): halves HBM traffic and
    doubles DVE throughput, 4x on the PE matmuls. Node-level softmax /
    normalization math stays f32.
"""

import os
import sys

for _p in ("/opt/trn_rl_repo", "/root/.axon_site/_ro/trn_rl_repo"):
    if os.path.isdir(_p) and _p not in sys.path:
        sys.path.insert(0, _p)

import numpy as np

import concourse.bacc as bacc
import concourse.bass as bass
import concourse.tile as tile
from concourse import mybir
from concourse.bass_utils import run_bass_kernel_spmd

F32 = mybir.dt.float32
F16 = mybir.dt.float16
BF16 = mybir.dt.bfloat16
AF = mybir.ActivationFunctionType
ALU = mybir.AluOpType
AX = mybir.AxisListType

N = 100000
E = 1600000
F_IN = 256
H1, D1 = 8, 8
HD1 = H1 * D1          # 64
D2 = 16                # H2 = 1
NEG = 0.2
NC = 8
P = 128
TILES = 784            # ceil(100000 / 128) rounded up to a multiple of 8
STEPS = TILES // NC    # 98
NPC = STEPS * P        # 12544 node rows handled per core in K1
PADS = -30000.0        # sentinel (fp16-safe): exp(lrelu(PADS + ad)) == 0

TRACE = False          # test.py flips this for NTFF profiling
SIM = False            # run through CoreSim instead of hardware
SIM_CORES = None       # e.g. [0] to only simulate core 0
LAST_EXEC_NS = []      # per-launch exec_time_ns when TRACE


def _run(nc, in_maps, tag):
    if SIM:
        from concourse.bass_interp import CoreSim

        outs = []
        cores = range(NC) if SIM_CORES is None else SIM_CORES
        for c in range(NC):
            if c not in cores:
                outs.append(outs[-1] if outs else {})
                continue
            sim = CoreSim(nc, trace=False)
            for k, v in in_maps[c].items():
                sim.tensor(k)[:] = v
            sim.simulate(check_with_hw=False)
            onames = [
                a.memorylocations[0].name
                for a in nc.m.functions[0].allocations
                if isinstance(a, mybir.MemoryLocationSet) and a.kind == "ExternalOutput"
            ]
            outs.append({k: np.array(sim.tensor(k)) for k in onames})
        return outs
    if TRACE:
        import hookfix  # noqa: F401  (registers antenv.axon_hooks)

        hookfix.install()
    res = run_bass_kernel_spmd(nc, in_maps, list(range(NC)), trace=TRACE)
    if TRACE:
        LAST_EXEC_NS.append((tag, res.exec_time_ns))
    return res.results


def _bc(ap, shape):
    """Broadcast the free dims of `ap` to `shape` (partition dim must already
    match).  Target dims are matched against source free dims right-to-left;
    size-1 source dims and unmatched target dims become step-0 (broadcast)."""
    src = ap.ap
    assert src[0][1] == shape[0], (src, shape)
    sdims = list(src[1:])
    res = []
    si = len(sdims) - 1
    for ti in range(len(shape) - 1, 0, -1):
        if si >= 0 and sdims[si][1] == shape[ti]:
            res.append(sdims[si])
            si -= 1
        elif si >= 0 and sdims[si][1] == 1:
            res.append([0, shape[ti]])
            si -= 1
        else:
            res.append([0, shape[ti]])
    assert si < 0, (src, shape)
    return bass.AP(tensor=ap.tensor, offset=ap.offset, ap=[src[0]] + res[::-1])


def _tail0(ap, n):
    """Append a trailing step-0 (broadcast) dim of size n."""
    return bass.AP(tensor=ap.tensor, offset=ap.offset, ap=list(ap.ap) + [[0, n]])


def _mid0(ap, pos, n):
    """Insert a step-0 (broadcast) dim of size n at free-dim position pos
    (ap.ap index pos, counting the partition dim as 0)."""
    dims = list(ap.ap)
    return bass.AP(
        tensor=ap.tensor, offset=ap.offset, ap=dims[:pos] + [[0, n]] + dims[pos:]
    )


def _tree_sum_k(nc, sl, out1, K):
    """Sum a [..., K] range over its trailing k axis via in-place halving
    tensor_tensor adds (2x fp16 DVE rate; tensor_reduce only streams at 1x).
    `sl(a, b)` must return the AP for the [..., a:b] k-slice; `out1` is the
    destination AP shaped like sl(0, 1)."""
    kc = K
    while kc > 2:
        h = kc // 2
        r = kc - h
        nc.vector.tensor_tensor(sl(0, h), sl(0, h), sl(r, r + h), op=ALU.add)
        kc = r
    if kc == 2:
        nc.vector.tensor_tensor(out1, sl(0, 1), sl(1, 2), op=ALU.add)
    else:
        nc.vector.tensor_copy(out1, sl(0, 1))


def _rep_row(nc, pool, dram_t, nparts, cols, tag, dtype=F32):
    """DMA-replicate a flat `cols`-element DRAM tensor across `nparts`
    partitions (engines cannot broadcast across partitions themselves)."""
    tl = pool.tile([nparts, cols], dtype, tag=tag)
    src = bass.AP(tensor=dram_t[:].tensor, offset=0, ap=[[0, nparts], [1, cols]])
    nc.sync.dma_start(tl[:], src)
    return tl


# --------------------------------------------------------------------------
# K1: node tables.  out column-major xq1T [80, NPC] fp16 per core:
#     rows 0:64 xp1 = x @ W1, 64:72 s1 (att_src dot), 72:80 ad1 (att_dst dot)
#   Input xh is host-laid-out [P, STEPS, 2, P]: xh[p, t, c, j] =
#   x[node t*128+j, feature c*128+p], so each group DMA reads one contiguous
#   multi-KB run per partition.
# --------------------------------------------------------------------------
def build_k1():
    nc = bacc.Bacc("TRN2", target_bir_lowering=False, debug=False, num_devices=NC)
    xh = nc.dram_tensor("xh", [P, STEPS, 2, P], F32, kind="ExternalInput")
    w1 = nc.dram_tensor("w1", [F_IN, HD1], F32, kind="ExternalInput")
    as1 = nc.dram_tensor("as1", [H1, D1], F32, kind="ExternalInput")
    ad1 = nc.dram_tensor("ad1", [H1, D1], F32, kind="ExternalInput")
    out = nc.dram_tensor("xq1T", [80, NPC], F16, kind="ExternalOutput")

    with tile.TileContext(nc) as tc:
        with (
            tc.tile_pool(name="pro", bufs=1) as pro,
            tc.tile_pool(name="io", bufs=3) as io,
            tc.tile_pool(name="ps", bufs=4, space="PSUM") as ps,
        ):
            w1sb = pro.tile([P, 2, HD1], F32)
            nc.sync.dma_start(w1sb[:], w1[:].rearrange("(c p) d -> p c d", p=P))
            asr = _rep_row(nc, pro, as1, P, HD1, "asr")
            adr = _rep_row(nc, pro, ad1, P, HD1, "adr")

            # w_s1[f, h] = sum_d W1[f, h*8+d] * att_src1[h, d]; same for dst
            wext = pro.tile([P, 2, 80], F32)
            nc.scalar.copy(wext[:, :, 0:HD1], w1sb[:])
            for att, lo in ((asr, 64), (adr, 72)):
                tmp = pro.tile([P, 2, HD1], F32, tag="k1tmp")
                nc.vector.tensor_tensor(
                    tmp[:], w1sb[:], _bc(att[:], [P, 2, HD1]), op=ALU.mult
                )
                nc.vector.tensor_reduce(
                    wext[:, :, lo : lo + 8],
                    tmp[:].rearrange("p c (h d) -> p c h d", d=D1),
                    axis=AX.X,
                    op=ALU.add,
                )
            wext16 = pro.tile([P, 2, 80], BF16)
            nc.scalar.copy(wext16[:], wext[:])

            GT = 8                                  # node-tiles per DMA group
            gi = 0
            for t0 in range(0, STEPS, GT):
                g = min(GT, STEPS - t0)
                qeng = nc.sync if gi % 2 == 0 else nc.scalar
                oeng = nc.scalar if gi % 2 == 0 else nc.sync
                gi += 1
                xt = io.tile([P, GT, 2, P], F32, tag="xt")
                qeng.dma_start(xt[:, 0:g], xh[:, t0 : t0 + g])
                xt16 = io.tile([P, GT, 2, P], BF16, tag="xt16")
                nc.vector.tensor_copy(xt16[:, 0:g], xt[:, 0:g])
                ot = io.tile([80, GT * P], F16, tag="k1o")
                for q0 in range(0, g, 4):           # 512-col psum chunks
                    gq = min(4, g - q0)
                    W = gq * P
                    pt = ps.tile([80, 4 * P], F32, tag="k1ps")
                    nc.tensor.matmul(
                        pt[:, 0:W],
                        lhsT=wext16[:, 0, :],
                        rhs=xt16[:, q0 : q0 + gq, 0, :],
                        start=True, stop=False,
                    )
                    nc.tensor.matmul(
                        pt[:, 0:W],
                        lhsT=wext16[:, 1, :],
                        rhs=xt16[:, q0 : q0 + gq, 1, :],
                        start=False, stop=True,
                    )
                    nc.scalar.copy(ot[:, q0 * P : q0 * P + W], pt[:, 0:W])
                oeng.dma_start(
                    out[:, t0 * P : (t0 + g) * P], ot[:, 0 : g * P]
                )
    nc.compile()
    return nc


# --------------------------------------------------------------------------
# K2: layer-1 edge aggregation + ELU + fused xp2/s2/ad2 table.
#   EV1 row (72 fp16): [xp1(64) | s1(8)] for the slot's src node (PADS rows
#   have s1 = -30000 so exp()==0).  p-major slots: slot = base + p*K + k.
#   out t2T [18, NPC] fp16 column-major: rows 0:16 xp2, 16 s2, 17 ad2.
# --------------------------------------------------------------------------
def build_k2(sched, chmax, moffsz):
    """V2a: edge slots on partitions.  Per 128-node tile t (tile max degree K,
    npc = 128//K nodes per chunk, nch chunks): chunk c holds the K-padded
    in-edges of nodes [c*npc, (c+1)*npc) on its 128 partitions.  Per chunk a
    single PE matmul with lhsT = [xpex(64,(d,h)) | ex(8)] and a constant 0/1
    mask as rhs scatter-aggregates numerators AND softmax denominators into a
    feature-major psum tile [72, 128 nodes] -- no DVE reduce at all.  Node-level
    softmax/ELU/W2 then run feature-major in 4-tile slabs."""
    slots = P * sum(nch for _, _, nch, _ in sched)
    S = 4                                   # tiles per node-op slab
    nc = bacc.Bacc("TRN2", target_bir_lowering=False, debug=False, num_devices=NC)
    ev = nc.dram_tensor("ev1", [80 * slots], F16, kind="ExternalInput")
    mk = nc.dram_tensor("masks", [P, moffsz], F16, kind="ExternalInput")
    rp = nc.dram_tensor("repmat", [H1, HD1], F16, kind="ExternalInput")
    w2p = nc.dram_tensor("w2p", [HD1, D2], F32, kind="ExternalInput")
    as2 = nc.dram_tensor("as2", [1, D2], F32, kind="ExternalInput")
    ad2 = nc.dram_tensor("ad2", [1, D2], F32, kind="ExternalInput")
    b1p = nc.dram_tensor("b1p", [HD1], F32, kind="ExternalInput")
    out = nc.dram_tensor("t2T", [18, NPC], F16, kind="ExternalOutput")

    with tile.TileContext(nc) as tc:
        with (
            nc.allow_low_precision(reason="fp16 pipeline, f32 where it matters"),
            tc.tile_pool(name="pro", bufs=1) as pro,
            tc.tile_pool(name="io", bufs=3) as io,
            tc.tile_pool(name="wk", bufs=2) as wk,
            tc.tile_pool(name="ps", bufs=2, space="PSUM") as ps,
        ):
            maskb = pro.tile([P, moffsz], F16)
            nc.sync.dma_start(maskb[:], mk[:])
            repb = pro.tile([H1, HD1], F16)
            nc.sync.dma_start(repb[:], rp[:])
            b1c = pro.tile([HD1, 1], F32)
            nc.sync.dma_start(b1c[:], b1p[:].rearrange("(p o) -> p o", o=1))
            w2sb = pro.tile([HD1, D2], F32)
            nc.sync.dma_start(w2sb[:], w2p[:])
            a2s = _rep_row(nc, pro, as2, HD1, D2, "a2s")
            a2d = _rep_row(nc, pro, ad2, HD1, D2, "a2d")

            # W2ext [64, 18] = [W2 | W2@att_src2 | W2@att_dst2], rows (d,h)
            w2e = pro.tile([HD1, 18], F32)
            nc.scalar.copy(w2e[:, 0:D2], w2sb[:])
            for att, col in ((a2s, 16), (a2d, 17)):
                tmp2 = pro.tile([HD1, D2], F32, tag="k2tmp")
                nc.vector.tensor_tensor(tmp2[:], w2sb[:], att[:], op=ALU.mult)
                nc.vector.tensor_reduce(
                    w2e[:, col : col + 1], tmp2[:], axis=AX.X, op=ALU.add
                )
            w2e16 = pro.tile([HD1, 18], F16)
            nc.scalar.copy(w2e16[:], w2e[:])
            c_eps = pro.tile([H1, 1], F32)
            nc.vector.memset(c_eps[:], 1e-4)

            base = 0
            nm16 = None
            for ti, (K, npc, nch, moff) in enumerate(sched):
                qeng = nc.sync if ti % 2 == 0 else nc.scalar
                oeng = nc.scalar if ti % 2 == 0 else nc.sync
                # per-slot row: [s1(8) | ad1(8) | xp1(64, (d,h))]
                evt = io.tile([P, chmax, 80], F16, tag="ev")
                qeng.dma_start(
                    evt[:, 0:nch, :],
                    ev[base : base + P * nch * 80].rearrange(
                        "(p c f) -> p c f", c=nch, f=80
                    ),
                )
                base += P * nch * 80

                e = wk.tile([P, chmax, H1], F16, tag="e")
                nc.vector.tensor_tensor(
                    e[:, 0:nch], evt[:, 0:nch, 0:8], evt[:, 0:nch, 8:16],
                    op=ALU.add,
                )
                ea = wk.tile([P, chmax, H1], F16, tag="ea")
                nc.vector.scalar_tensor_tensor(
                    ea[:, 0:nch], e[:, 0:nch], NEG, e[:, 0:nch],
                    op0=ALU.mult, op1=ALU.max,
                )
                lhsT = wk.tile([P, chmax, 72], F16, tag="lhsT")
                nc.scalar.activation(lhsT[:, 0:nch, 64:72], ea[:, 0:nch], AF.Exp)
                nc.vector.tensor_tensor(
                    lhsT[:, 0:nch, 0:64].rearrange("p c (d h) -> p c d h", h=H1),
                    evt[:, 0:nch, 16:80].rearrange("p c (d h) -> p c d h", h=H1),
                    _mid0(lhsT[:, 0:nch, 64:72], 2, D1),
                    op=ALU.mult,
                )

                # scatter-aggregate: num[(d,h), node] rows 0:64, dn[h, node] 64:72
                pnum = ps.tile([72, P], F32, tag="pnum")
                for c in range(nch):
                    w = min(npc, P - c * npc)
                    nc.tensor.matmul(
                        pnum[:, c * npc : c * npc + w],
                        lhsT=lhsT[:, c, :],
                        rhs=maskb[:, moff : moff + w],
                        start=True, stop=True,
                    )
                si = ti % S
                if si == 0:
                    nm16 = wk.tile([72, S, P], F16, tag="nm16")
                nc.scalar.copy(nm16[:, si, :], pnum[:])

                if si == S - 1 or ti == len(sched) - 1:
                    ns = si + 1
                    t0 = ti - ns + 1
                    # inv = 1/(dn+eps), replicated across the 8 d-rows via PE
                    dne = wk.tile([H1, S, P], F16, tag="dne")
                    nc.scalar.activation(
                        dne[:, 0:ns], nm16[64:72, 0:ns], AF.Identity,
                        bias=c_eps[:],
                    )
                    inv32 = wk.tile([H1, S, P], F32, tag="inv32")
                    nc.vector.reciprocal(inv32[:, 0:ns], dne[:, 0:ns])
                    inv16 = wk.tile([H1, S, P], F16, tag="inv16")
                    nc.scalar.copy(inv16[:, 0:ns], inv32[:, 0:ns])
                    pir = ps.tile([HD1, S, P], F32, tag="pir")
                    nc.tensor.matmul(
                        pir[:, 0:ns, :].rearrange("f s n -> f (s n)"),
                        lhsT=repb[:],
                        rhs=inv16[:, 0:ns, :].rearrange("h s n -> h (s n)"),
                        start=True, stop=True,
                    )
                    ir16 = wk.tile([HD1, S, P], F16, tag="ir16")
                    nc.scalar.copy(ir16[:, 0:ns], pir[:, 0:ns])
                    # h = elu(num*inv + b1)
                    h = wk.tile([HD1, S, P], F16, tag="h")
                    nc.vector.tensor_tensor(
                        h[:, 0:ns], nm16[0:64, 0:ns], ir16[:, 0:ns], op=ALU.mult
                    )
                    nc.scalar.activation(
                        h[:, 0:ns], h[:, 0:ns], AF.Identity, bias=b1c[:]
                    )
                    hpos = wk.tile([HD1, S, P], F16, tag="hpos")
                    nc.vector.tensor_scalar_max(hpos[:, 0:ns], h[:, 0:ns], 0.0)
                    nc.vector.tensor_scalar_min(h[:, 0:ns], h[:, 0:ns], 0.0)
                    exq = wk.tile([HD1, S, P], F16, tag="exq")
                    nc.scalar.activation(exq[:, 0:ns], h[:, 0:ns], AF.Exp)
                    h16 = wk.tile([HD1, S, P], F16, tag="h16")
                    nc.vector.scalar_tensor_tensor(
                        h16[:, 0:ns], exq[:, 0:ns], -1.0, hpos[:, 0:ns],
                        op0=ALU.add, op1=ALU.add,
                    )
                    # t2 = W2ext^T @ h  (feature-major: no transposes needed)
                    pt2 = ps.tile([18, S, P], F32, tag="pt2")
                    nc.tensor.matmul(
                        pt2[:, 0:ns, :].rearrange("r s n -> r (s n)"),
                        lhsT=w2e16[:],
                        rhs=h16[:, 0:ns, :].rearrange("f s n -> f (s n)"),
                        start=True, stop=True,
                    )
                    st2 = io.tile([18, S, P], F16, tag="st2")
                    nc.scalar.copy(st2[:, 0:ns], pt2[:, 0:ns])
                    oeng.dma_start(
                        out[:, t0 * P : (ti + 1) * P],
                        st2[:, 0:ns].rearrange("r s n -> r (s n)"),
                    )
    nc.compile()
    return nc


def build_k3(groups):
    tot = 17 * P * sum(g * kb for _, g, kb in groups)
    nc = bacc.Bacc("TRN2", target_bir_lowering=False, debug=False, num_devices=NC)
    ev = nc.dram_tensor("ev2", [tot], F16, kind="ExternalInput")
    adt = nc.dram_tensor("ad2R", [P, STEPS], F16, kind="ExternalInput")
    b2t = nc.dram_tensor("b2", [D2], F32, kind="ExternalInput")
    out = nc.dram_tensor("o3", [NPC, D2], F32, kind="ExternalOutput")

    with tile.TileContext(nc) as tc:
        with (
            nc.allow_low_precision(reason="fp16 pipeline, f32 where it matters"),
            tc.tile_pool(name="pro", bufs=1) as pro,
            tc.tile_pool(name="io", bufs=3) as io,
            tc.tile_pool(name="wk", bufs=2) as wk,
        ):
            b2r = _rep_row(nc, pro, b2t, P, D2, "b2r")
            adfull = pro.tile([P, STEPS], F16)
            nc.sync.dma_start(adfull[:], adt[:])
            # persistent per-node buffers: log_softmax finalized in one pass
            obuf = pro.tile([P, STEPS, D2], F32)
            ssbuf = pro.tile([P, STEPS], F32)

            base = 0
            gi = 0
            for t0, G, K in groups:
                qeng = nc.sync if gi % 2 == 0 else nc.scalar
                gi += 1
                evt = io.tile([P, G, 17 * K], F16, tag="ev")
                qeng.dma_start(
                    evt[:],
                    ev[base : base + P * G * 17 * K].rearrange(
                        "(p g f) -> p g f", g=G, f=17 * K
                    ),
                )
                base += P * G * 17 * K

                # e = s2 + ad2;  ex = exp(lrelu(e))
                e = wk.tile([P, G, K], F16, tag="e")
                nc.vector.tensor_tensor(
                    e[:], evt[:, :, 16 * K : 17 * K],
                    _tail0(adfull[:, t0 : t0 + G], K), op=ALU.add,
                )
                ea = wk.tile([P, G, K], F16, tag="ea")
                nc.vector.scalar_tensor_tensor(
                    ea[:], e[:], NEG, e[:], op0=ALU.mult, op1=ALU.max
                )
                ex = wk.tile([P, G, K], F16, tag="ex")
                nc.scalar.activation(ex[:], ea[:], AF.Exp)

                dn = wk.tile([P, G], F16, tag="dn")
                nc.vector.tensor_reduce(dn[:], ex[:], axis=AX.X, op=ALU.add)
                nc.vector.tensor_scalar_add(dn[:], dn[:], 1e-4)
                inv = wk.tile([P, G], F16, tag="inv")
                nc.vector.reciprocal(inv[:], dn[:])

                msg = wk.tile([P, G, D2, K], F16, tag="msg")
                nc.vector.tensor_tensor(
                    msg[:],
                    evt[:, :, 0 : 16 * K].rearrange("p g (d k) -> p g d k", k=K),
                    _mid0(ex[:], 2, D2),
                    op=ALU.mult,
                )
                oacc = wk.tile([P, G, D2], F16, tag="oacc")
                nc.vector.tensor_reduce(oacc[:], msg[:], axis=AX.X, op=ALU.add)
                o = obuf[:, t0 : t0 + G, :]
                nc.vector.tensor_tensor(
                    o, oacc[:], _tail0(inv[:], D2), op=ALU.mult
                )
                nc.vector.tensor_tensor(o, o, _bc(b2r[:], [P, G, D2]), op=ALU.add)

                # log_softmax part 1: subtract max, exp, accumulate denominator
                nm = wk.tile([P, G], F32, tag="nm")
                nc.vector.tensor_reduce(nm, o, axis=AX.X, op=ALU.max, negate=True)
                nc.vector.tensor_tensor(o, o, _tail0(nm[:], D2), op=ALU.add)
                exq = wk.tile([P, G, D2], F32, tag="exq")
                nc.scalar.activation(exq[:], o, AF.Exp)
                nc.vector.tensor_reduce(
                    ssbuf[:, t0 : t0 + G], exq[:], axis=AX.X, op=ALU.add
                )

            # log_softmax part 2: one Ln over all nodes, one subtract, one DMA
            lss = pro.tile([P, STEPS], F32)
            nc.scalar.activation(lss[:], ssbuf[:], AF.Ln)
            nc.vector.tensor_tensor(
                obuf[:], obuf[:], _tail0(lss[:], D2), op=ALU.subtract
            )
            nc.sync.dma_start(
                out[:].rearrange("(t p) f -> p t f", p=P), obuf[:]
            )
    nc.compile()
    return nc


# --------------------------------------------------------------------------
# Host orchestration
# --------------------------------------------------------------------------
def _make_groups(k_step, gmax, slot_budget):
    """Greedy: grow the group while tiles*K stays under slot_budget."""
    groups = []
    t0 = 0
    while t0 < STEPS:
        g = 1
        kb = max(int(k_step[t0]), 1)
        while (
            t0 + g < STEPS
            and g < gmax
            and (g + 1) * max(kb, int(k_step[t0 + g])) <= slot_budget
        ):
            kb = max(kb, int(k_step[t0 + g]))
            g += 1
        groups.append((t0, g, kb))
        t0 += g
    return groups


def _build_slots(groups, spos_node, deg, estart, src_by_dst):
    """slot -> src node id (N = pad) per core; layout per group is p-major:
    slot = base + p*(G*K) + g*K + k."""
    tot = sum(P * g * kb for _, g, kb in groups)
    slot = np.full((NC, tot), N, dtype=np.int64)
    arangeP = np.arange(P)
    for c in range(NC):
        base = 0
        for t0, g, kb in groups:
            for gi in range(g):
                T = (t0 + gi) * NC + c
                nodes = spos_node[T * P : (T + 1) * P]
                valid = nodes >= 0
                nv = nodes[valid]
                if nv.size == 0:
                    continue
                d = deg[nv]
                rowstart = base + arangeP[valid] * (g * kb) + gi * kb
                totd = int(d.sum())
                if totd == 0:
                    continue
                rep_row = np.repeat(rowstart, d)
                rep_cum = np.repeat(np.cumsum(d) - d, d)
                intra = np.arange(totd) - rep_cum
                rep_est = np.repeat(estart[nv], d)
                slot[c, rep_row + intra] = src_by_dst[rep_est + intra]
            base += P * g * kb
    return slot


def kernel(x, edge_index, W1, att_src1, att_dst1, b1, W2, att_src2, att_dst2, b2):
    x = np.asarray(x, dtype=np.float32)
    edge_index = np.asarray(edge_index)
    W1 = np.asarray(W1, dtype=np.float32)
    att_src1 = np.asarray(att_src1, dtype=np.float32)
    att_dst1 = np.asarray(att_dst1, dtype=np.float32)
    b1 = np.asarray(b1, dtype=np.float32)
    W2 = np.asarray(W2, dtype=np.float32)
    att_src2 = np.asarray(att_src2, dtype=np.float32).reshape(1, D2)
    att_dst2 = np.asarray(att_dst2, dtype=np.float32).reshape(1, D2)
    b2 = np.asarray(b2, dtype=np.float32)

    src = edge_index[0].astype(np.int64)
    dst = edge_index[1].astype(np.int64)

    # ---- schedule: degree-sorted tiles, round-robin dealt across cores ----
    deg = np.bincount(dst, minlength=N)
    order = np.argsort(deg, kind="stable")          # sorted-node space -> node id
    eo = np.argsort(dst, kind="stable")             # edges sorted by dst
    src_by_dst = src[eo]
    estart = np.zeros(N + 1, dtype=np.int64)
    estart[1:] = np.cumsum(deg)

    spos_node = np.full(TILES * P, -1, dtype=np.int64)
    spos_node[:N] = order
    sdeg = np.zeros(TILES * P, dtype=np.int64)
    sdeg[:N] = deg[order]
    tile_max = sdeg.reshape(TILES, P).max(axis=1)
    k_step = np.maximum(tile_max.reshape(STEPS, NC).max(axis=1), 1)  # [STEPS]
    k_step = ((k_step + 1) // 2) * 2       # even K: keeps fp16 rows 4B-aligned


    # ---- V2a schedule for K2: slots on partitions, whole nodes per chunk ----
    Ks = sorted(set(int(k) for k in k_step))
    mask_off = {}
    moffsz = 0
    for Kv in Ks:
        mask_off[Kv] = moffsz
        moffsz += P // Kv
    maskbuf = np.zeros((P, moffsz), dtype=np.float16)
    pidx_all = np.arange(P)
    for Kv in Ks:
        o = mask_off[Kv]
        npc = P // Kv
        col = pidx_all // Kv
        v = col < npc
        maskbuf[pidx_all[v], o + col[v]] = 1.0
    sched = []
    for t in range(STEPS):
        Kv = int(k_step[t])
        npc = P // Kv
        nch = -(-P // npc)
        sched.append((Kv, npc, nch, mask_off[Kv]))
    chmax = max(n for _, _, n, _ in sched)
    repmat = np.zeros((H1, HD1), dtype=np.float16)
    for dd in range(D1):
        for hh in range(H1):
            repmat[hh, dd * H1 + hh] = 1.0
    perm = np.arange(HD1).reshape(H1, D1).T.ravel()  # row (d*8+h) <- h*8+d

    groups3 = _make_groups(k_step, 8, 200)
    slots3 = _build_slots(groups3, spos_node, deg, estart, src_by_dst)
    ad_rows = np.where(spos_node < 0, N, spos_node)  # [TILES*P] node per row
    # per-core view: row t*128+p of core c <-> sorted pos (t*NC+c)*128+p
    ad_rows = (
        ad_rows.reshape(STEPS, NC, P).transpose(1, 0, 2).reshape(NC, NPC)
    )

    # ---- K1: node tables ----
    xpad = np.zeros((NC * NPC, F_IN), dtype=np.float32)
    xpad[:N] = x
    nc1 = build_k1()
    in1 = [
        {
            # xh[p, t, c, j] = x[node t*128+j, feature c*128+p]
            "xh": np.ascontiguousarray(
                xpad[c * NPC : (c + 1) * NPC]
                .T.reshape(2, P, STEPS, P)
                .transpose(1, 2, 0, 3)
            ),
            "w1": W1,
            "as1": att_src1,
            "ad1": att_dst1,
        }
        for c in range(NC)
    ]
    r1 = _run(nc1, in1, "k1")
    xq1 = np.empty((NC * NPC + 1, 80), dtype=np.float16)
    for c in range(NC):
        xq1[c * NPC : (c + 1) * NPC] = r1[c]["xq1T"].T
    xq1[-1] = 0.0
    xq1[-1, 64:72] = PADS                           # pad row: s1 = -30000

    # ---- K2: layer 1 ----
    nc2 = build_k2(sched, chmax, moffsz)

    def _ev2a(c):
        """Per tile: [nch chunks x 128 slots x 80] rows, p-major on disk:
        row = [s1(8) | ad1-of-dst(8) | xp1(64, (d,h) order)]."""
        padrow = NC * NPC
        blocks = []
        for t in range(STEPS):
            Kv, npc, nch, _ = sched[t]
            T = t * NC + c
            nodes = spos_node[T * P : (T + 1) * P]
            col = pidx_all // Kv
            kk = pidx_all % Kv
            nit = np.arange(nch)[:, None] * npc + col[None, :]
            validc = (col < npc)[None, :] & (nit < P)
            nodearr = np.where(validc, nodes[np.clip(nit, 0, P - 1)], -1)
            dstrow = np.where(nodearr >= 0, nodearr, padrow)
            nsafe = np.clip(nodearr, 0, N - 1)
            hasedge = (nodearr >= 0) & (kk[None, :] < deg[nsafe])
            eidx = estart[nsafe] + kk[None, :]
            srcn = np.where(hasedge, src_by_dst[np.clip(eidx, 0, E - 1)], padrow)
            rows = xq1[srcn]                                  # [nch, P, 80]
            blk = np.empty((nch, P, 80), dtype=np.float16)
            blk[..., 0:8] = rows[..., 64:72]
            blk[..., 8:16] = xq1[dstrow, 72:80]
            blk[..., 16:80] = (
                rows[..., 0:64]
                .reshape(nch, P, 8, 8)
                .transpose(0, 1, 3, 2)
                .reshape(nch, P, 64)
            )
            blocks.append(blk.transpose(1, 0, 2).ravel())
        return np.concatenate(blocks)

    in2 = [
        {
            "ev1": _ev2a(c),
            "masks": maskbuf,
            "repmat": repmat,
            "w2p": W2[perm],
            "as2": att_src2,
            "ad2": att_dst2,
            "b1p": b1[perm],
        }
        for c in range(NC)
    ]
    r2 = _run(nc2, in2, "k2")

    # reassemble layer-2 node table in original-node space
    t2 = np.zeros((N + 1, 18), dtype=np.float16)
    t2[N, 16] = PADS                                # pad row: s2 = -30000
    for c in range(NC):
        cols = r2[c]["t2T"]                         # [18, NPC] fp16
        rows = cols.T.reshape(STEPS, P, 18)
        for t in range(STEPS):
            T = t * NC + c
            nodes = spos_node[T * P : (T + 1) * P]
            valid = nodes >= 0
            t2[nodes[valid]] = rows[t][valid]

    # ---- K3: layer 2 ----
    nc3 = build_k3(groups3)
    pad3 = np.where(slots3 >= N, N, slots3)

    def _soa2(c):
        """Per (group, p, g) blocks: [xp2 (16,K) | s2 (K)], k innermost."""
        rows = t2[pad3[c]]
        out = np.empty(rows.shape[0] * 17, dtype=np.float16)
        bs = 0
        bf = 0
        for _t0, g, kb in groups3:
            n = P * g * kb
            arr = rows[bs : bs + n].reshape(P, g, kb, 18)
            xp = arr[..., 0:16].transpose(0, 1, 3, 2).reshape(P, g, 16 * kb)
            s = arr[..., 16].reshape(P, g, kb)
            out[bf : bf + n * 17] = np.concatenate([xp, s], axis=2).ravel()
            bs += n
            bf += n * 17
        return out

    in3 = [
        {
            "ev2": _soa2(c),
            "ad2R": np.ascontiguousarray(
                t2[np.where(ad_rows[c] >= N, N, ad_rows[c]), 17]
                .reshape(STEPS, P)
                .T
            ),
            "b2": b2,
        }
        for c in range(NC)
    ]
    r3 = _run(nc3, in3, "k3")

    outp = np.zeros((N, D2), dtype=np.float32)
    for c in range(NC):
        o = r3[c]["o3"].reshape(STEPS, P, D2)
        for t in range(STEPS):
            T = t * NC + c
            nodes = spos_node[T * P : (T + 1) * P]
            valid = nodes >= 0
            outp[nodes[valid]] = o[t][valid]
    return outp


# revision 33
# speedup vs baseline: 1.2268x; 1.2268x over previous
"""GAT 2-layer network on 8 Trainium2 NeuronCores.

Strategy (edge-parallel, per the sharding hint "partition edges, replicate
node features"):
  - Nodes are sorted by in-degree and packed into 128-node tiles; tiles are
    dealt round-robin onto the 8 cores so every core runs the identical
    instruction stream (SPMD) over a shared per-step K schedule.
  - All FLOPs run on device across 3 launches:
      K1: xp1 = x @ W1 plus per-head attention dot products (s1, ad1).
      K2: per dst-tile segment softmax + message aggregation for layer 1,
          ELU, then xp2 = h @ W2ext (fused) -> layer-2 node table.
      K3: layer-2 segment softmax + aggregation + bias + log_softmax.
  - Between launches the host only does index-based data movement: it
    replicates the device-computed per-node tables into per-edge-slot
    streams (degree-padded, p-major) so each device step reads purely
    sequential DMA. No floating-point math happens on the host.
  - The edge streams and all bulk elementwise work run in fp16 (device
    casts on device; host only moves fp16 bytes): halves HBM traffic and
    doubles DVE throughput, 4x on the PE matmuls. Node-level softmax /
    normalization math stays f32.
"""

import os
import sys

for _p in ("/opt/trn_rl_repo", "/root/.axon_site/_ro/trn_rl_repo"):
    if os.path.isdir(_p) and _p not in sys.path:
        sys.path.insert(0, _p)

import numpy as np

import concourse.bacc as bacc
import concourse.bass as bass
import concourse.tile as tile
from concourse import mybir
from concourse.bass_utils import run_bass_kernel_spmd

F32 = mybir.dt.float32
F16 = mybir.dt.float16
BF16 = mybir.dt.bfloat16
AF = mybir.ActivationFunctionType
ALU = mybir.AluOpType
AX = mybir.AxisListType

N = 100000
E = 1600000
F_IN = 256
H1, D1 = 8, 8
HD1 = H1 * D1          # 64
D2 = 16                # H2 = 1
NEG = 0.2
NC = 8
P = 128
TILES = 784            # ceil(100000 / 128) rounded up to a multiple of 8
STEPS = TILES // NC    # 98
NPC = STEPS * P        # 12544 node rows handled per core in K1
PADS = -30000.0        # sentinel (fp16-safe): exp(lrelu(PADS + ad)) == 0

TRACE = False          # test.py flips this for NTFF profiling
SIM = False            # run through CoreSim instead of hardware
SIM_CORES = None       # e.g. [0] to only simulate core 0
LAST_EXEC_NS = []      # per-launch exec_time_ns when TRACE


def _run(nc, in_maps, tag):
    if SIM:
        from concourse.bass_interp import CoreSim

        outs = []
        cores = range(NC) if SIM_CORES is None else SIM_CORES
        for c in range(NC):
            if c not in cores:
                outs.append(outs[-1] if outs else {})
                continue
            sim = CoreSim(nc, trace=False)
            for k, v in in_maps[c].items():
                sim.tensor(k)[:] = v
            sim.simulate(check_with_hw=False)
            onames = [
                a.memorylocations[0].name
                for a in nc.m.functions[0].allocations
                if isinstance(a, mybir.MemoryLocationSet) and a.kind == "ExternalOutput"
            ]
            outs.append({k: np.array(sim.tensor(k)) for k in onames})
        return outs
    if TRACE:
        import hookfix  # noqa: F401  (registers antenv.axon_hooks)

        hookfix.install()
    res = run_bass_kernel_spmd(nc, in_maps, list(range(NC)), trace=TRACE)
    if TRACE:
        LAST_EXEC_NS.append((tag, res.exec_time_ns))
    return res.results


def _bc(ap, shape):
    """Broadcast the free dims of `ap` to `shape` (partition dim must already
    match).  Target dims are matched against source free dims right-to-left;
    size-1 source dims and unmatched target dims become step-0 (broadcast)."""
    src = ap.ap
    assert src[0][1] == shape[0], (src, shape)
    sdims = list(src[1:])
    res = []
    si = len(sdims) - 1
    for ti in range(len(shape) - 1, 0, -1):
        if si >= 0 and sdims[si][1] == shape[ti]:
            res.append(sdims[si])
            si -= 1
        elif si >= 0 and sdims[si][1] == 1:
            res.append([0, shape[ti]])
            si -= 1
        else:
            res.append([0, shape[ti]])
    assert si < 0, (src, shape)
    return bass.AP(tensor=ap.tensor, offset=ap.offset, ap=[src[0]] + res[::-1])


def _tail0(ap, n):
    """Append a trailing step-0 (broadcast) dim of size n."""
    return bass.AP(tensor=ap.tensor, offset=ap.offset, ap=list(ap.ap) + [[0, n]])


def _mid0(ap, pos, n):
    """Insert a step-0 (broadcast) dim of size n at free-dim position pos
    (ap.ap index pos, counting the partition dim as 0)."""
    dims = list(ap.ap)
    return bass.AP(
        tensor=ap.tensor, offset=ap.offset, ap=dims[:pos] + [[0, n]] + dims[pos:]
    )


def _tree_sum_k(nc, sl, out1, K, sl2=None):
    """Sum a [..., K] range over its trailing k axis via halving tensor_tensor
    adds (2x fp16 DVE rate; tensor_reduce only streams at 1x).  `sl(a, b)`
    returns the AP for the [..., a:b] k-slice; `sl2` is an equally-shaped
    scratch (ping-pong, avoids in-place read-modify-write); `out1` is the
    destination AP shaped like sl(0, 1)."""
    kc = K
    cur, nxt = sl, (sl2 if sl2 is not None else sl)
    while kc > 2:
        h = (kc // 2) & ~1          # even slice sizes keep 4B alignment
        r = kc - h
        nc.vector.tensor_tensor(nxt(0, h), cur(0, h), cur(r, r + h), op=ALU.add)
        if r > h and nxt is not cur:
            nc.vector.tensor_copy(nxt(h, r), cur(h, r))
        kc = r
        cur, nxt = nxt, cur
    if kc == 2:
        nc.vector.tensor_tensor(out1, cur(0, 1), cur(1, 2), op=ALU.add)
    else:
        nc.vector.tensor_copy(out1, cur(0, 1))


def _rep_row(nc, pool, dram_t, nparts, cols, tag, dtype=F32):
    """DMA-replicate a flat `cols`-element DRAM tensor across `nparts`
    partitions (engines cannot broadcast across partitions themselves)."""
    tl = pool.tile([nparts, cols], dtype, tag=tag)
    src = bass.AP(tensor=dram_t[:].tensor, offset=0, ap=[[0, nparts], [1, cols]])
    nc.sync.dma_start(tl[:], src)
    return tl


# --------------------------------------------------------------------------
# K1: node tables.  out column-major xq1T [80, NPC] fp16 per core:
#     rows 0:64 xp1 = x @ W1, 64:72 s1 (att_src dot), 72:80 ad1 (att_dst dot)
#   Input xh is host-laid-out [P, STEPS, 2, P]: xh[p, t, c, j] =
#   x[node t*128+j, feature c*128+p], so each group DMA reads one contiguous
#   multi-KB run per partition.
# --------------------------------------------------------------------------
def build_k1():
    nc = bacc.Bacc("TRN2", target_bir_lowering=False, debug=False, num_devices=NC)
    xh = nc.dram_tensor("xh", [P, STEPS, 2, P], F32, kind="ExternalInput")
    w1 = nc.dram_tensor("w1", [F_IN, HD1], F32, kind="ExternalInput")
    as1 = nc.dram_tensor("as1", [H1, D1], F32, kind="ExternalInput")
    ad1 = nc.dram_tensor("ad1", [H1, D1], F32, kind="ExternalInput")
    out = nc.dram_tensor("xq1T", [80, NPC], F16, kind="ExternalOutput")

    with tile.TileContext(nc) as tc:
        with (
            tc.tile_pool(name="pro", bufs=1) as pro,
            tc.tile_pool(name="io", bufs=3) as io,
            tc.tile_pool(name="ps", bufs=4, space="PSUM") as ps,
        ):
            w1sb = pro.tile([P, 2, HD1], F32)
            nc.sync.dma_start(w1sb[:], w1[:].rearrange("(c p) d -> p c d", p=P))
            asr = _rep_row(nc, pro, as1, P, HD1, "asr")
            adr = _rep_row(nc, pro, ad1, P, HD1, "adr")

            # w_s1[f, h] = sum_d W1[f, h*8+d] * att_src1[h, d]; same for dst
            wext = pro.tile([P, 2, 80], F32)
            nc.scalar.copy(wext[:, :, 0:HD1], w1sb[:])
            for att, lo in ((asr, 64), (adr, 72)):
                tmp = pro.tile([P, 2, HD1], F32, tag="k1tmp")
                nc.vector.tensor_tensor(
                    tmp[:], w1sb[:], _bc(att[:], [P, 2, HD1]), op=ALU.mult
                )
                nc.vector.tensor_reduce(
                    wext[:, :, lo : lo + 8],
                    tmp[:].rearrange("p c (h d) -> p c h d", d=D1),
                    axis=AX.X,
                    op=ALU.add,
                )
            wext16 = pro.tile([P, 2, 80], BF16)
            nc.scalar.copy(wext16[:], wext[:])

            GT = 8                                  # node-tiles per DMA group
            gi = 0
            for t0 in range(0, STEPS, GT):
                g = min(GT, STEPS - t0)
                qeng = nc.sync if gi % 2 == 0 else nc.scalar
                oeng = nc.scalar if gi % 2 == 0 else nc.sync
                gi += 1
                xt = io.tile([P, GT, 2, P], F32, tag="xt")
                qeng.dma_start(xt[:, 0:g], xh[:, t0 : t0 + g])
                xt16 = io.tile([P, GT, 2, P], BF16, tag="xt16")
                nc.vector.tensor_copy(xt16[:, 0:g], xt[:, 0:g])
                ot = io.tile([80, GT * P], F16, tag="k1o")
                for q0 in range(0, g, 4):           # 512-col psum chunks
                    gq = min(4, g - q0)
                    W = gq * P
                    pt = ps.tile([80, 4 * P], F32, tag="k1ps")
                    nc.tensor.matmul(
                        pt[:, 0:W],
                        lhsT=wext16[:, 0, :],
                        rhs=xt16[:, q0 : q0 + gq, 0, :],
                        start=True, stop=False,
                    )
                    nc.tensor.matmul(
                        pt[:, 0:W],
                        lhsT=wext16[:, 1, :],
                        rhs=xt16[:, q0 : q0 + gq, 1, :],
                        start=False, stop=True,
                    )
                    nc.scalar.copy(ot[:, q0 * P : q0 * P + W], pt[:, 0:W])
                oeng.dma_start(
                    out[:, t0 * P : (t0 + g) * P], ot[:, 0 : g * P]
                )
    nc.compile()
    return nc


# --------------------------------------------------------------------------
# K2: layer-1 edge aggregation + ELU + fused xp2/s2/ad2 table.
#   EV1 row (72 fp16): [xp1(64) | s1(8)] for the slot's src node (PADS rows
#   have s1 = -30000 so exp()==0).  p-major slots: slot = base + p*K + k.
#   out t2T [18, NPC] fp16 column-major: rows 0:16 xp2, 16 s2, 17 ad2.
# --------------------------------------------------------------------------
def build_k2(groups, k_tile):
    slots = P * sum(g * kb for _, g, kb in groups)
    nc = bacc.Bacc("TRN2", target_bir_lowering=False, debug=False, num_devices=NC)
    ev = nc.dram_tensor("ev1", [72 * slots], F16, kind="ExternalInput")
    adt = nc.dram_tensor("adR", [P, STEPS, H1], F16, kind="ExternalInput")
    w2 = nc.dram_tensor("w2", [HD1, D2], F32, kind="ExternalInput")
    as2 = nc.dram_tensor("as2", [1, D2], F32, kind="ExternalInput")
    ad2 = nc.dram_tensor("ad2", [1, D2], F32, kind="ExternalInput")
    b1t = nc.dram_tensor("b1", [HD1], F32, kind="ExternalInput")
    out = nc.dram_tensor("t2T", [18, NPC], F16, kind="ExternalOutput")

    from concourse.masks import make_identity

    with tile.TileContext(nc) as tc:
        with (
            nc.allow_low_precision(reason="fp16 pipeline, f32 where it matters"),
            tc.tile_pool(name="pro", bufs=1) as pro,
            tc.tile_pool(name="io", bufs=2) as io,
            tc.tile_pool(name="wk", bufs=2) as wk,
            tc.tile_pool(name="ps", bufs=2, space="PSUM") as ps,
        ):
            w2sb = pro.tile([HD1, D2], F32)
            nc.sync.dma_start(w2sb[:], w2[:])
            a2s = _rep_row(nc, pro, as2, HD1, D2, "a2s")
            a2d = _rep_row(nc, pro, ad2, HD1, D2, "a2d")
            b1r = _rep_row(nc, pro, b1t, P, HD1, "b1r")
            b1r16 = pro.tile([P, HD1], F16)
            nc.vector.tensor_copy(b1r16[:], b1r[:])
            c_zero = pro.tile([P, 1], F16)
            nc.vector.memset(c_zero[:], 0.0)
            adfull = pro.tile([P, STEPS, H1], F16)
            nc.sync.dma_start(adfull[:], adt[:])
            ident = pro.tile([P, P], F16)
            make_identity(nc, ident[:])

            # W2ext [64, 18] = [W2 | W2@att_src2 | W2@att_dst2]
            w2e = pro.tile([HD1, 18], F32)
            nc.scalar.copy(w2e[:, 0:D2], w2sb[:])
            for att, col in ((a2s, 16), (a2d, 17)):
                tmp2 = pro.tile([HD1, D2], F32, tag="k2tmp")
                nc.vector.tensor_tensor(tmp2[:], w2sb[:], att[:], op=ALU.mult)
                nc.vector.tensor_reduce(
                    w2e[:, col : col + 1], tmp2[:], axis=AX.X, op=ALU.add
                )
            w2e16 = pro.tile([HD1, 18], F16)
            nc.scalar.copy(w2e16[:], w2e[:])

            base = 0
            gi = 0
            for t0, G, K in groups:
                qeng = nc.sync if gi % 2 == 0 else nc.scalar
                oeng = nc.scalar if gi % 2 == 0 else nc.sync
                gi += 1
                # one merged stream per partition: [s1 (G,8,K) | xp1 (G,8,8,K)]
                evt = io.tile([P, 72 * G * K], F16, tag="ev")
                qeng.dma_start(
                    evt[:],
                    ev[72 * base : 72 * (base + P * G * K)].rearrange(
                        "(p f) -> p f", f=72 * G * K
                    ),
                )
                base += P * G * K
                est = evt[:, 0 : 8 * G * K].rearrange(
                    "p (g h k) -> p g h k", h=H1, k=K
                )
                xpall = evt[:, 8 * G * K : 72 * G * K].rearrange(
                    "p (gh d k) -> p gh d k", d=D1, k=K
                )
                adv = adfull[:, t0 : t0 + G, :]

                # e = s1 + ad1 (GpSimd);  ex = exp(lrelu(e))  (lrelu DVE, exp Act)
                e = wk.tile([P, G, H1, K], F16, tag="e")
                nc.vector.tensor_tensor(e[:], est, _tail0(adv, K), op=ALU.add)
                ea = wk.tile([P, G, H1, K], F16, tag="ea")
                nc.vector.scalar_tensor_tensor(
                    ea[:], e[:], NEG, e[:], op0=ALU.mult, op1=ALU.max
                )
                ex = wk.tile([P, G, H1, K], F16, tag="ex")
                nc.scalar.activation(ex[:], ea[:], AF.Exp)

                # denom + reciprocal
                dn = wk.tile([P, G, H1], F16, tag="dn")
                nc.vector.tensor_reduce(dn[:], ex[:], axis=AX.X, op=ALU.add)
                nc.vector.tensor_scalar_add(dn[:], dn[:], 1e-4)
                inv = wk.tile([P, G, H1], F16, tag="inv")
                nc.vector.reciprocal(inv[:], dn[:])

                # msg[p,(g h),d,k] = ex * xp; agg = tree-sum over k  (fp16)
                msg = wk.tile([P, G * H1, D1, K], F16, tag="msg")
                exall = ex[:].rearrange("p g h k -> p (g h) k")
                nc.vector.tensor_tensor(
                    msg[:], xpall, _mid0(exall[:], 2, D1), op=ALU.mult
                )
                agg = wk.tile([P, G, H1, D1, 1], F16, tag="agg")
                _tree_sum_k(
                    nc, lambda a, b: msg[:, :, :, a:b],
                    agg[:].rearrange("p g h d o -> p (g h) d o"), K,
                )

                # h = elu(agg * inv + b1)   (fp16; exp on Act, minmax GpSimd)
                hb = wk.tile([P, G, HD1], F16, tag="hb")
                nc.vector.tensor_tensor(
                    hb[:].rearrange("p g (h d) -> p g h d", d=D1),
                    agg[:, :, :, :, 0],
                    _tail0(inv[:], D1),
                    op=ALU.mult,
                )
                nc.vector.tensor_tensor(
                    hb[:], hb[:], _bc(b1r16[:], [P, G, HD1]), op=ALU.add
                )
                hpos = wk.tile([P, G, HD1], F16, tag="hpos")
                nc.vector.tensor_scalar_max(hpos[:], hb[:], 0.0)
                hneg = wk.tile([P, G, HD1], F16, tag="hneg")
                nc.vector.tensor_scalar_min(hneg[:], hb[:], 0.0)
                exq = wk.tile([P, G, HD1], F16, tag="exq")
                nc.scalar.activation(exq[:], hneg[:], AF.Exp)
                h16 = wk.tile([P, G, HD1], F16, tag="h16")
                nc.vector.scalar_tensor_tensor(
                    h16[:], exq[:], -1.0, hpos[:], op0=ALU.add, op1=ALU.add
                )

                # xp2/s2/ad2 via per-tile transpose + matmul (fp16)
                shT = wk.tile([HD1, G, P], F16, tag="shT")
                pt2 = ps.tile([18, G, P], F32, tag="pt2")
                for g in range(G):
                    phT = ps.tile([HD1, P], F16, tag="phT")
                    nc.tensor.transpose(phT[:], h16[:, g, :], ident[:])
                    nc.scalar.copy(shT[:, g, :], phT[:])
                    nc.tensor.matmul(
                        pt2[:, g, :], lhsT=w2e16[:], rhs=shT[:, g, :],
                        start=True, stop=True,
                    )
                st2 = io.tile([18, G, P], F16, tag="st2")
                nc.scalar.copy(st2[:], pt2[:])
                oeng.dma_start(
                    out[:, t0 * P : (t0 + G) * P],
                    st2[:].rearrange("r g n -> r (g n)"),
                )
    nc.compile()
    return nc


# --------------------------------------------------------------------------
# K3: layer-2 edge aggregation + bias + log_softmax.
#   EV2 row (17 fp16): [xp2(16) | s2(1)] for the slot's src node.
# --------------------------------------------------------------------------
def build_k3(groups):
    tot = 17 * P * sum(g * kb for _, g, kb in groups)
    nc = bacc.Bacc("TRN2", target_bir_lowering=False, debug=False, num_devices=NC)
    ev = nc.dram_tensor("ev2", [tot], F16, kind="ExternalInput")
    adt = nc.dram_tensor("ad2R", [P, STEPS], F16, kind="ExternalInput")
    b2t = nc.dram_tensor("b2", [D2], F32, kind="ExternalInput")
    out = nc.dram_tensor("o3", [NPC, D2], F32, kind="ExternalOutput")

    with tile.TileContext(nc) as tc:
        with (
            nc.allow_low_precision(reason="fp16 pipeline, f32 where it matters"),
            tc.tile_pool(name="pro", bufs=1) as pro,
            tc.tile_pool(name="io", bufs=3) as io,
            tc.tile_pool(name="wk", bufs=2) as wk,
        ):
            b2r = _rep_row(nc, pro, b2t, P, D2, "b2r")
            adfull = pro.tile([P, STEPS], F16)
            nc.sync.dma_start(adfull[:], adt[:])
            # persistent per-node buffers: log_softmax finalized in one pass
            obuf = pro.tile([P, STEPS, D2], F32)
            ssbuf = pro.tile([P, STEPS], F32)

            base = 0
            gi = 0
            for t0, G, K in groups:
                qeng = nc.sync if gi % 2 == 0 else nc.scalar
                gi += 1
                evt = io.tile([P, G, 17 * K], F16, tag="ev")
                qeng.dma_start(
                    evt[:],
                    ev[base : base + P * G * 17 * K].rearrange(
                        "(p g f) -> p g f", g=G, f=17 * K
                    ),
                )
                base += P * G * 17 * K

                # e = s2 + ad2;  ex = exp(lrelu(e))
                e = wk.tile([P, G, K], F16, tag="e")
                nc.vector.tensor_tensor(
                    e[:], evt[:, :, 16 * K : 17 * K],
                    _tail0(adfull[:, t0 : t0 + G], K), op=ALU.add,
                )
                ea = wk.tile([P, G, K], F16, tag="ea")
                nc.vector.scalar_tensor_tensor(
                    ea[:], e[:], NEG, e[:], op0=ALU.mult, op1=ALU.max
                )
                ex = wk.tile([P, G, K], F16, tag="ex")
                nc.scalar.activation(ex[:], ea[:], AF.Exp)

                dn = wk.tile([P, G], F16, tag="dn")
                nc.vector.tensor_reduce(dn[:], ex[:], axis=AX.X, op=ALU.add)
                nc.vector.tensor_scalar_add(dn[:], dn[:], 1e-4)
                inv = wk.tile([P, G], F16, tag="inv")
                nc.vector.reciprocal(inv[:], dn[:])

                msg = wk.tile([P, G, D2, K], F16, tag="msg")
                nc.vector.tensor_tensor(
                    msg[:],
                    evt[:, :, 0 : 16 * K].rearrange("p g (d k) -> p g d k", k=K),
                    _mid0(ex[:], 2, D2),
                    op=ALU.mult,
                )
                oacc = wk.tile([P, G, D2, 1], F16, tag="oacc")
                _tree_sum_k(nc, lambda a, b: msg[:, :, :, a:b], oacc[:], K)
                o = obuf[:, t0 : t0 + G, :]
                nc.vector.tensor_tensor(
                    o, oacc[:, :, :, 0], _tail0(inv[:], D2), op=ALU.mult
                )
                nc.vector.tensor_tensor(o, o, _bc(b2r[:], [P, G, D2]), op=ALU.add)

                # log_softmax part 1: subtract max, exp, accumulate denominator
                nm = wk.tile([P, G], F32, tag="nm")
                nc.vector.tensor_reduce(nm, o, axis=AX.X, op=ALU.max, negate=True)
                nc.vector.tensor_tensor(o, o, _tail0(nm[:], D2), op=ALU.add)
                exq = wk.tile([P, G, D2], F32, tag="exq")
                nc.scalar.activation(exq[:], o, AF.Exp)
                nc.vector.tensor_reduce(
                    ssbuf[:, t0 : t0 + G], exq[:], axis=AX.X, op=ALU.add
                )

            # log_softmax part 2: one Ln over all nodes, one subtract, one DMA
            lss = pro.tile([P, STEPS], F32)
            nc.scalar.activation(lss[:], ssbuf[:], AF.Ln)
            nc.vector.tensor_tensor(
                obuf[:], obuf[:], _tail0(lss[:], D2), op=ALU.subtract
            )
            nc.sync.dma_start(
                out[:].rearrange("(t p) f -> p t f", p=P), obuf[:]
            )
    nc.compile()
    return nc


# --------------------------------------------------------------------------
# Host orchestration
# --------------------------------------------------------------------------
def _make_groups(k_step, gmax, slot_budget):
    """Greedy: grow the group while tiles*K stays under slot_budget."""
    groups = []
    t0 = 0
    while t0 < STEPS:
        g = 1
        kb = max(int(k_step[t0]), 1)
        while (
            t0 + g < STEPS
            and g < gmax
            and (g + 1) * max(kb, int(k_step[t0 + g])) <= slot_budget
        ):
            kb = max(kb, int(k_step[t0 + g]))
            g += 1
        groups.append((t0, g, kb))
        t0 += g
    return groups


def _build_slots(groups, spos_node, deg, estart, src_by_dst):
    """slot -> src node id (N = pad) per core; layout per group is p-major:
    slot = base + p*(G*K) + g*K + k."""
    tot = sum(P * g * kb for _, g, kb in groups)
    slot = np.full((NC, tot), N, dtype=np.int64)
    arangeP = np.arange(P)
    for c in range(NC):
        base = 0
        for t0, g, kb in groups:
            for gi in range(g):
                T = (t0 + gi) * NC + c
                nodes = spos_node[T * P : (T + 1) * P]
                valid = nodes >= 0
                nv = nodes[valid]
                if nv.size == 0:
                    continue
                d = deg[nv]
                rowstart = base + arangeP[valid] * (g * kb) + gi * kb
                totd = int(d.sum())
                if totd == 0:
                    continue
                rep_row = np.repeat(rowstart, d)
                rep_cum = np.repeat(np.cumsum(d) - d, d)
                intra = np.arange(totd) - rep_cum
                rep_est = np.repeat(estart[nv], d)
                slot[c, rep_row + intra] = src_by_dst[rep_est + intra]
            base += P * g * kb
    return slot


def kernel(x, edge_index, W1, att_src1, att_dst1, b1, W2, att_src2, att_dst2, b2):
    x = np.asarray(x, dtype=np.float32)
    edge_index = np.asarray(edge_index)
    W1 = np.asarray(W1, dtype=np.float32)
    att_src1 = np.asarray(att_src1, dtype=np.float32)
    att_dst1 = np.asarray(att_dst1, dtype=np.float32)
    b1 = np.asarray(b1, dtype=np.float32)
    W2 = np.asarray(W2, dtype=np.float32)
    att_src2 = np.asarray(att_src2, dtype=np.float32).reshape(1, D2)
    att_dst2 = np.asarray(att_dst2, dtype=np.float32).reshape(1, D2)
    b2 = np.asarray(b2, dtype=np.float32)

    src = edge_index[0].astype(np.int64)
    dst = edge_index[1].astype(np.int64)

    # ---- schedule: degree-sorted tiles, round-robin dealt across cores ----
    deg = np.bincount(dst, minlength=N)
    order = np.argsort(deg, kind="stable")          # sorted-node space -> node id
    eo = np.argsort(dst, kind="stable")             # edges sorted by dst
    src_by_dst = src[eo]
    estart = np.zeros(N + 1, dtype=np.int64)
    estart[1:] = np.cumsum(deg)

    spos_node = np.full(TILES * P, -1, dtype=np.int64)
    spos_node[:N] = order
    sdeg = np.zeros(TILES * P, dtype=np.int64)
    sdeg[:N] = deg[order]
    tile_max = sdeg.reshape(TILES, P).max(axis=1)
    k_step = np.maximum(tile_max.reshape(STEPS, NC).max(axis=1), 1)  # [STEPS]
    k_step = ((k_step + 1) // 2) * 2       # even K: keeps fp16 rows 4B-aligned


    groups2 = _make_groups(k_step, 4, 96)
    groups3 = _make_groups(k_step, 8, 200)
    slots2 = _build_slots(groups2, spos_node, deg, estart, src_by_dst)
    slots3 = _build_slots(groups3, spos_node, deg, estart, src_by_dst)
    ad_rows = np.where(spos_node < 0, N, spos_node)  # [TILES*P] node per row
    # per-core view: row t*128+p of core c <-> sorted pos (t*NC+c)*128+p
    ad_rows = (
        ad_rows.reshape(STEPS, NC, P).transpose(1, 0, 2).reshape(NC, NPC)
    )

    # ---- K1: node tables ----
    xpad = np.zeros((NC * NPC, F_IN), dtype=np.float32)
    xpad[:N] = x
    nc1 = build_k1()
    in1 = [
        {
            # xh[p, t, c, j] = x[node t*128+j, feature c*128+p]
            "xh": np.ascontiguousarray(
                xpad[c * NPC : (c + 1) * NPC]
                .T.reshape(2, P, STEPS, P)
                .transpose(1, 2, 0, 3)
            ),
            "w1": W1,
            "as1": att_src1,
            "ad1": att_dst1,
        }
        for c in range(NC)
    ]
    r1 = _run(nc1, in1, "k1")
    xq1 = np.empty((NC * NPC + 1, 80), dtype=np.float16)
    for c in range(NC):
        xq1[c * NPC : (c + 1) * NPC] = r1[c]["xq1T"].T
    xq1[-1] = 0.0
    xq1[-1, 64:72] = PADS                           # pad row: s1 = -30000

    # ---- K2: layer 1 ----
    nc2 = build_k2(groups2, k_step)
    pad2 = np.where(slots2 >= N, NC * NPC, slots2)

    def _soa1(c):
        """One merged stream, per (group, p, g) blocks, k innermost:
        [s1 (8,K) | xp1 (8,8,K)]."""
        rows = xq1[pad2[c], 0:72]
        outc = np.empty(rows.shape[0] * 72, dtype=np.float16)
        bs = 0
        for _t0, g, kb in groups2:
            n = P * g * kb
            arr = rows[bs : bs + n].reshape(P, g, kb, 72)
            s = arr[..., 64:72].transpose(0, 1, 3, 2).reshape(P, g * 8 * kb)
            xp = (
                arr[..., 0:64]
                .reshape(P, g, kb, 8, 8)
                .transpose(0, 1, 3, 4, 2)
                .reshape(P, g * 64 * kb)
            )
            outc[bs * 72 : (bs + n) * 72] = np.concatenate([s, xp], axis=1).ravel()
            bs += n
        return outc

    in2 = [
        {
            "ev1": _soa1(c),
            "adR": np.ascontiguousarray(
                xq1[np.where(ad_rows[c] >= N, NC * NPC, ad_rows[c]), 72:80]
                .reshape(STEPS, P, H1)
                .transpose(1, 0, 2)
            ),
            "w2": W2,
            "as2": att_src2,
            "ad2": att_dst2,
            "b1": b1,
        }
        for c in range(NC)
    ]
    r2 = _run(nc2, in2, "k2")

    # reassemble layer-2 node table in original-node space
    t2 = np.zeros((N + 1, 18), dtype=np.float16)
    t2[N, 16] = PADS                                # pad row: s2 = -30000
    for c in range(NC):
        cols = r2[c]["t2T"]                         # [18, NPC] fp16
        rows = cols.T.reshape(STEPS, P, 18)
        for t in range(STEPS):
            T = t * NC + c
            nodes = spos_node[T * P : (T + 1) * P]
            valid = nodes >= 0
            t2[nodes[valid]] = rows[t][valid]

    # ---- K3: layer 2 ----
    nc3 = build_k3(groups3)
    pad3 = np.where(slots3 >= N, N, slots3)

    def _soa2(c):
        """Per (group, p, g) blocks: [xp2 (16,K) | s2 (K)], k innermost."""
        rows = t2[pad3[c]]
        out = np.empty(rows.shape[0] * 17, dtype=np.float16)
        bs = 0
        bf = 0
        for _t0, g, kb in groups3:
            n = P * g * kb
            arr = rows[bs : bs + n].reshape(P, g, kb, 18)
            xp = arr[..., 0:16].transpose(0, 1, 3, 2).reshape(P, g, 16 * kb)
            s = arr[..., 16].reshape(P, g, kb)
            out[bf : bf + n * 17] = np.concatenate([xp, s], axis=2).ravel()
            bs += n
            bf += n * 17
        return out

    in3 = [
        {
            "ev2": _soa2(c),
            "ad2R": np.ascontiguousarray(
                t2[np.where(ad_rows[c] >= N, N, ad_rows[c]), 17]
                .reshape(STEPS, P)
                .T
            ),
            "b2": b2,
        }
        for c in range(NC)
    ]
    r3 = _run(nc3, in3, "k3")

    outp = np.zeros((N, D2), dtype=np.float32)
    for c in range(NC):
        o = r3[c]["o3"].reshape(STEPS, P, D2)
        for t in range(STEPS):
            T = t * NC + c
            nodes = spos_node[T * P : (T + 1) * P]
            valid = nodes >= 0
            outp[nodes[valid]] = o[t][valid]
    return outp


# revision 35
# speedup vs baseline: 1.2607x; 1.0276x over previous
"""GAT 2-layer network on 8 Trainium2 NeuronCores.

Strategy (edge-parallel, per the sharding hint "partition edges, replicate
node features"):
  - Nodes are sorted by in-degree and packed into 128-node tiles; tiles are
    dealt round-robin onto the 8 cores so every core runs the identical
    instruction stream (SPMD) over a shared per-step K schedule.
  - All FLOPs run on device across 3 launches:
      K1: xp1 = x @ W1 plus per-head attention dot products (s1, ad1).
      K2: per dst-tile segment softmax + message aggregation for layer 1,
          ELU, then xp2 = h @ W2ext (fused) -> layer-2 node table.
      K3: layer-2 segment softmax + aggregation + bias + log_softmax.
  - Between launches the host only does index-based data movement: it
    replicates the device-computed per-node tables into per-edge-slot
    streams (degree-padded, p-major) so each device step reads purely
    sequential DMA. No floating-point math happens on the host.
  - The edge streams and all bulk elementwise work run in fp16 (device
    casts on device; host only moves fp16 bytes): halves HBM traffic and
    doubles DVE throughput, 4x on the PE matmuls. Node-level softmax /
    normalization math stays f32.
"""

import os
import sys

for _p in ("/opt/trn_rl_repo", "/root/.axon_site/_ro/trn_rl_repo"):
    if os.path.isdir(_p) and _p not in sys.path:
        sys.path.insert(0, _p)

import numpy as np

import concourse.bacc as bacc
import concourse.bass as bass
import concourse.tile as tile
from concourse import mybir
from concourse.bass_utils import run_bass_kernel_spmd

F32 = mybir.dt.float32
F16 = mybir.dt.float16
BF16 = mybir.dt.bfloat16
AF = mybir.ActivationFunctionType
ALU = mybir.AluOpType
AX = mybir.AxisListType

N = 100000
E = 1600000
F_IN = 256
H1, D1 = 8, 8
HD1 = H1 * D1          # 64
D2 = 16                # H2 = 1
NEG = 0.2
NC = 8
P = 128
TILES = 784            # ceil(100000 / 128) rounded up to a multiple of 8
STEPS = TILES // NC    # 98
NPC = STEPS * P        # 12544 node rows handled per core in K1
PADS = -30000.0        # sentinel (fp16-safe): exp(lrelu(PADS + ad)) == 0

TRACE = False          # test.py flips this for NTFF profiling
SIM = False            # run through CoreSim instead of hardware
SIM_CORES = None       # e.g. [0] to only simulate core 0
LAST_EXEC_NS = []      # per-launch exec_time_ns when TRACE


def _run(nc, in_maps, tag):
    if SIM:
        from concourse.bass_interp import CoreSim

        outs = []
        cores = range(NC) if SIM_CORES is None else SIM_CORES
        for c in range(NC):
            if c not in cores:
                outs.append(outs[-1] if outs else {})
                continue
            sim = CoreSim(nc, trace=False)
            for k, v in in_maps[c].items():
                sim.tensor(k)[:] = v
            sim.simulate(check_with_hw=False)
            onames = [
                a.memorylocations[0].name
                for a in nc.m.functions[0].allocations
                if isinstance(a, mybir.MemoryLocationSet) and a.kind == "ExternalOutput"
            ]
            outs.append({k: np.array(sim.tensor(k)) for k in onames})
        return outs
    if TRACE:
        import hookfix  # noqa: F401  (registers antenv.axon_hooks)

        hookfix.install()
    res = run_bass_kernel_spmd(nc, in_maps, list(range(NC)), trace=TRACE)
    if TRACE:
        LAST_EXEC_NS.append((tag, res.exec_time_ns))
    return res.results


def _bc(ap, shape):
    """Broadcast the free dims of `ap` to `shape` (partition dim must already
    match).  Target dims are matched against source free dims right-to-left;
    size-1 source dims and unmatched target dims become step-0 (broadcast)."""
    src = ap.ap
    assert src[0][1] == shape[0], (src, shape)
    sdims = list(src[1:])
    res = []
    si = len(sdims) - 1
    for ti in range(len(shape) - 1, 0, -1):
        if si >= 0 and sdims[si][1] == shape[ti]:
            res.append(sdims[si])
            si -= 1
        elif si >= 0 and sdims[si][1] == 1:
            res.append([0, shape[ti]])
            si -= 1
        else:
            res.append([0, shape[ti]])
    assert si < 0, (src, shape)
    return bass.AP(tensor=ap.tensor, offset=ap.offset, ap=[src[0]] + res[::-1])


def _tail0(ap, n):
    """Append a trailing step-0 (broadcast) dim of size n."""
    return bass.AP(tensor=ap.tensor, offset=ap.offset, ap=list(ap.ap) + [[0, n]])


def _mid0(ap, pos, n):
    """Insert a step-0 (broadcast) dim of size n at free-dim position pos
    (ap.ap index pos, counting the partition dim as 0)."""
    dims = list(ap.ap)
    return bass.AP(
        tensor=ap.tensor, offset=ap.offset, ap=dims[:pos] + [[0, n]] + dims[pos:]
    )


def _tree_sum_k(nc, sl, out1, K, sl2=None):
    """Sum a [..., K] range over its trailing k axis via halving tensor_tensor
    adds (2x fp16 DVE rate; tensor_reduce only streams at 1x).  `sl(a, b)`
    returns the AP for the [..., a:b] k-slice; `sl2` is an equally-shaped
    scratch (ping-pong, avoids in-place read-modify-write); `out1` is the
    destination AP shaped like sl(0, 1)."""
    kc = K
    cur, nxt = sl, (sl2 if sl2 is not None else sl)
    while kc > 2:
        h = (kc // 2) & ~1          # even slice sizes keep 4B alignment
        r = kc - h
        nc.vector.tensor_tensor(nxt(0, h), cur(0, h), cur(r, r + h), op=ALU.add)
        if r > h and nxt is not cur:
            nc.vector.tensor_copy(nxt(h, r), cur(h, r))
        kc = r
        cur, nxt = nxt, cur
    if kc == 2:
        nc.vector.tensor_tensor(out1, cur(0, 1), cur(1, 2), op=ALU.add)
    else:
        nc.vector.tensor_copy(out1, cur(0, 1))


def _rep_row(nc, pool, dram_t, nparts, cols, tag, dtype=F32):
    """DMA-replicate a flat `cols`-element DRAM tensor across `nparts`
    partitions (engines cannot broadcast across partitions themselves)."""
    tl = pool.tile([nparts, cols], dtype, tag=tag)
    src = bass.AP(tensor=dram_t[:].tensor, offset=0, ap=[[0, nparts], [1, cols]])
    nc.sync.dma_start(tl[:], src)
    return tl


# --------------------------------------------------------------------------
# K1: node tables.  out column-major xq1T [80, NPC] fp16 per core:
#     rows 0:64 xp1 = x @ W1, 64:72 s1 (att_src dot), 72:80 ad1 (att_dst dot)
#   Input xh is host-laid-out [P, STEPS, 2, P]: xh[p, t, c, j] =
#   x[node t*128+j, feature c*128+p], so each group DMA reads one contiguous
#   multi-KB run per partition.
# --------------------------------------------------------------------------
def build_k1():
    nc = bacc.Bacc("TRN2", target_bir_lowering=False, debug=False, num_devices=NC)
    xh = nc.dram_tensor("xh", [P, STEPS, 2, P], F32, kind="ExternalInput")
    w1 = nc.dram_tensor("w1", [F_IN, HD1], F32, kind="ExternalInput")
    as1 = nc.dram_tensor("as1", [H1, D1], F32, kind="ExternalInput")
    ad1 = nc.dram_tensor("ad1", [H1, D1], F32, kind="ExternalInput")
    out = nc.dram_tensor("xq1T", [80, NPC], F16, kind="ExternalOutput")

    with tile.TileContext(nc) as tc:
        with (
            tc.tile_pool(name="pro", bufs=1) as pro,
            tc.tile_pool(name="io", bufs=3) as io,
            tc.tile_pool(name="ps", bufs=4, space="PSUM") as ps,
        ):
            w1sb = pro.tile([P, 2, HD1], F32)
            nc.sync.dma_start(w1sb[:], w1[:].rearrange("(c p) d -> p c d", p=P))
            asr = _rep_row(nc, pro, as1, P, HD1, "asr")
            adr = _rep_row(nc, pro, ad1, P, HD1, "adr")

            # w_s1[f, h] = sum_d W1[f, h*8+d] * att_src1[h, d]; same for dst
            wext = pro.tile([P, 2, 80], F32)
            nc.scalar.copy(wext[:, :, 0:HD1], w1sb[:])
            for att, lo in ((asr, 64), (adr, 72)):
                tmp = pro.tile([P, 2, HD1], F32, tag="k1tmp")
                nc.vector.tensor_tensor(
                    tmp[:], w1sb[:], _bc(att[:], [P, 2, HD1]), op=ALU.mult
                )
                nc.vector.tensor_reduce(
                    wext[:, :, lo : lo + 8],
                    tmp[:].rearrange("p c (h d) -> p c h d", d=D1),
                    axis=AX.X,
                    op=ALU.add,
                )
            wext16 = pro.tile([P, 2, 80], BF16)
            nc.scalar.copy(wext16[:], wext[:])

            GT = 8                                  # node-tiles per DMA group
            gi = 0
            for t0 in range(0, STEPS, GT):
                g = min(GT, STEPS - t0)
                qeng = nc.sync if gi % 2 == 0 else nc.scalar
                oeng = nc.scalar if gi % 2 == 0 else nc.sync
                gi += 1
                xt = io.tile([P, GT, 2, P], F32, tag="xt")
                qeng.dma_start(xt[:, 0:g], xh[:, t0 : t0 + g])
                xt16 = io.tile([P, GT, 2, P], BF16, tag="xt16")
                nc.vector.tensor_copy(xt16[:, 0:g], xt[:, 0:g])
                ot = io.tile([80, GT * P], F16, tag="k1o")
                for q0 in range(0, g, 4):           # 512-col psum chunks
                    gq = min(4, g - q0)
                    W = gq * P
                    pt = ps.tile([80, 4 * P], F32, tag="k1ps")
                    nc.tensor.matmul(
                        pt[:, 0:W],
                        lhsT=wext16[:, 0, :],
                        rhs=xt16[:, q0 : q0 + gq, 0, :],
                        start=True, stop=False,
                    )
                    nc.tensor.matmul(
                        pt[:, 0:W],
                        lhsT=wext16[:, 1, :],
                        rhs=xt16[:, q0 : q0 + gq, 1, :],
                        start=False, stop=True,
                    )
                    nc.scalar.copy(ot[:, q0 * P : q0 * P + W], pt[:, 0:W])
                oeng.dma_start(
                    out[:, t0 * P : (t0 + g) * P], ot[:, 0 : g * P]
                )
    nc.compile()
    return nc


# --------------------------------------------------------------------------
# K2: layer-1 edge aggregation + ELU + fused xp2/s2/ad2 table.
#   EV1 row (72 fp16): [xp1(64) | s1(8)] for the slot's src node (PADS rows
#   have s1 = -30000 so exp()==0).  p-major slots: slot = base + p*K + k.
#   out t2T [18, NPC] fp16 column-major: rows 0:16 xp2, 16 s2, 17 ad2.
# --------------------------------------------------------------------------
def build_k2(groups, k_tile):
    slots = P * sum(g * kb for _, g, kb in groups)
    nc = bacc.Bacc("TRN2", target_bir_lowering=False, debug=False, num_devices=NC)
    ev = nc.dram_tensor("ev1", [80 * slots], F16, kind="ExternalInput")
    w2 = nc.dram_tensor("w2", [HD1, D2], F32, kind="ExternalInput")
    as2 = nc.dram_tensor("as2", [1, D2], F32, kind="ExternalInput")
    ad2 = nc.dram_tensor("ad2", [1, D2], F32, kind="ExternalInput")
    b1t = nc.dram_tensor("b1", [HD1], F32, kind="ExternalInput")
    out = nc.dram_tensor("t2T", [18, NPC], F16, kind="ExternalOutput")

    from concourse.masks import make_identity

    with tile.TileContext(nc) as tc:
        with (
            nc.allow_low_precision(reason="fp16 pipeline, f32 where it matters"),
            tc.tile_pool(name="pro", bufs=1) as pro,
            tc.tile_pool(name="io", bufs=2) as io,
            tc.tile_pool(name="wk", bufs=2) as wk,
            tc.tile_pool(name="ps", bufs=2, space="PSUM") as ps,
        ):
            w2sb = pro.tile([HD1, D2], F32)
            nc.sync.dma_start(w2sb[:], w2[:])
            a2s = _rep_row(nc, pro, as2, HD1, D2, "a2s")
            a2d = _rep_row(nc, pro, ad2, HD1, D2, "a2d")
            b1r = _rep_row(nc, pro, b1t, P, HD1, "b1r")
            b1r16 = pro.tile([P, HD1], F16)
            nc.vector.tensor_copy(b1r16[:], b1r[:])
            c_zero = pro.tile([P, 1], F16)
            nc.vector.memset(c_zero[:], 0.0)
            ident = pro.tile([P, P], F16)
            make_identity(nc, ident[:])

            # W2ext [64, 18] = [W2 | W2@att_src2 | W2@att_dst2]
            w2e = pro.tile([HD1, 18], F32)
            nc.scalar.copy(w2e[:, 0:D2], w2sb[:])
            for att, col in ((a2s, 16), (a2d, 17)):
                tmp2 = pro.tile([HD1, D2], F32, tag="k2tmp")
                nc.vector.tensor_tensor(tmp2[:], w2sb[:], att[:], op=ALU.mult)
                nc.vector.tensor_reduce(
                    w2e[:, col : col + 1], tmp2[:], axis=AX.X, op=ALU.add
                )
            w2e16 = pro.tile([HD1, 18], F16)
            nc.scalar.copy(w2e16[:], w2e[:])

            base = 0
            gi = 0
            for t0, G, K in groups:
                qeng = nc.sync if gi % 2 == 0 else nc.scalar
                oeng = nc.scalar if gi % 2 == 0 else nc.sync
                gi += 1
                # merged stream per partition: [s1 | ad1-per-slot | xp1]
                evt = io.tile([P, 80 * G * K], F16, tag="ev")
                qeng.dma_start(
                    evt[:],
                    ev[80 * base : 80 * (base + P * G * K)].rearrange(
                        "(p f) -> p f", f=80 * G * K
                    ),
                )
                base += P * G * K
                est = evt[:, 0 : 8 * G * K].rearrange(
                    "p (g h k) -> p g h k", h=H1, k=K
                )
                adv2 = evt[:, 8 * G * K : 16 * G * K].rearrange(
                    "p (g h k) -> p g h k", h=H1, k=K
                )
                xpall = evt[:, 16 * G * K : 80 * G * K].rearrange(
                    "p (gh d k) -> p gh d k", d=D1, k=K
                )

                # e = s1 + ad1 (dense+dense: 2x);  ex = exp(lrelu(e))
                e = wk.tile([P, G, H1, K], F16, tag="e")
                nc.vector.tensor_tensor(e[:], est, adv2, op=ALU.add)
                ea = wk.tile([P, G, H1, K], F16, tag="ea")
                nc.vector.scalar_tensor_tensor(
                    ea[:], e[:], NEG, e[:], op0=ALU.mult, op1=ALU.max
                )
                ex = wk.tile([P, G, H1, K], F16, tag="ex")
                nc.scalar.activation(ex[:], ea[:], AF.Exp)

                # denom + reciprocal
                dn = wk.tile([P, G, H1], F16, tag="dn")
                nc.vector.tensor_reduce(dn[:], ex[:], axis=AX.X, op=ALU.add)
                nc.vector.tensor_scalar_add(dn[:], dn[:], 1e-4)
                inv = wk.tile([P, G, H1], F16, tag="inv")
                nc.vector.reciprocal(inv[:], dn[:])

                # msg[p,(g h),d,k] = ex * xp; agg = tree-sum over k  (fp16)
                msg = wk.tile([P, G * H1, D1, K], F16, tag="msg")
                exall = ex[:].rearrange("p g h k -> p (g h) k")
                nc.vector.tensor_tensor(
                    msg[:], xpall, _mid0(exall[:], 2, D1), op=ALU.mult
                )
                agg = wk.tile([P, G, H1, D1, 1], F16, tag="agg")
                _tree_sum_k(
                    nc, lambda a, b: msg[:, :, :, a:b],
                    agg[:].rearrange("p g h d o -> p (g h) d o"), K,
                )

                # h = elu(agg * inv + b1)   (fp16; exp on Act, minmax GpSimd)
                hb = wk.tile([P, G, HD1], F16, tag="hb")
                nc.vector.tensor_tensor(
                    hb[:].rearrange("p g (h d) -> p g h d", d=D1),
                    agg[:, :, :, :, 0],
                    _tail0(inv[:], D1),
                    op=ALU.mult,
                )
                nc.vector.tensor_tensor(
                    hb[:], hb[:], _bc(b1r16[:], [P, G, HD1]), op=ALU.add
                )
                hpos = wk.tile([P, G, HD1], F16, tag="hpos")
                nc.vector.tensor_scalar_max(hpos[:], hb[:], 0.0)
                hneg = wk.tile([P, G, HD1], F16, tag="hneg")
                nc.vector.tensor_scalar_min(hneg[:], hb[:], 0.0)
                exq = wk.tile([P, G, HD1], F16, tag="exq")
                nc.scalar.activation(exq[:], hneg[:], AF.Exp)
                h16 = wk.tile([P, G, HD1], F16, tag="h16")
                nc.vector.scalar_tensor_tensor(
                    h16[:], exq[:], -1.0, hpos[:], op0=ALU.add, op1=ALU.add
                )

                # xp2/s2/ad2 via per-tile transpose + matmul (fp16)
                shT = wk.tile([HD1, G, P], F16, tag="shT")
                pt2 = ps.tile([18, G, P], F32, tag="pt2")
                for g in range(G):
                    phT = ps.tile([HD1, P], F16, tag="phT")
                    nc.tensor.transpose(phT[:], h16[:, g, :], ident[:])
                    nc.scalar.copy(shT[:, g, :], phT[:])
                    nc.tensor.matmul(
                        pt2[:, g, :], lhsT=w2e16[:], rhs=shT[:, g, :],
                        start=True, stop=True,
                    )
                st2 = io.tile([18, G, P], F16, tag="st2")
                nc.scalar.copy(st2[:], pt2[:])
                oeng.dma_start(
                    out[:, t0 * P : (t0 + G) * P],
                    st2[:].rearrange("r g n -> r (g n)"),
                )
    nc.compile()
    return nc


# --------------------------------------------------------------------------
# K3: layer-2 edge aggregation + bias + log_softmax.
#   EV2 row (17 fp16): [xp2(16) | s2(1)] for the slot's src node.
# --------------------------------------------------------------------------
def build_k3(groups):
    tot = 18 * P * sum(g * kb for _, g, kb in groups)
    nc = bacc.Bacc("TRN2", target_bir_lowering=False, debug=False, num_devices=NC)
    ev = nc.dram_tensor("ev2", [tot], F16, kind="ExternalInput")
    b2t = nc.dram_tensor("b2", [D2], F32, kind="ExternalInput")
    out = nc.dram_tensor("o3", [NPC, D2], F32, kind="ExternalOutput")

    with tile.TileContext(nc) as tc:
        with (
            nc.allow_low_precision(reason="fp16 pipeline, f32 where it matters"),
            tc.tile_pool(name="pro", bufs=1) as pro,
            tc.tile_pool(name="io", bufs=3) as io,
            tc.tile_pool(name="wk", bufs=2) as wk,
        ):
            b2r = _rep_row(nc, pro, b2t, P, D2, "b2r")
            # persistent per-node buffers: log_softmax finalized in one pass
            obuf = pro.tile([P, STEPS, D2], F32)
            ssbuf = pro.tile([P, STEPS], F32)

            base = 0
            gi = 0
            for t0, G, K in groups:
                qeng = nc.sync if gi % 2 == 0 else nc.scalar
                gi += 1
                evt = io.tile([P, G, 18 * K], F16, tag="ev")
                qeng.dma_start(
                    evt[:],
                    ev[base : base + P * G * 18 * K].rearrange(
                        "(p g f) -> p g f", g=G, f=18 * K
                    ),
                )
                base += P * G * 18 * K

                # e = s2 + ad2 (dense+dense: 2x);  ex = exp(lrelu(e))
                e = wk.tile([P, G, K], F16, tag="e")
                nc.vector.tensor_tensor(
                    e[:], evt[:, :, 16 * K : 17 * K],
                    evt[:, :, 17 * K : 18 * K], op=ALU.add,
                )
                ea = wk.tile([P, G, K], F16, tag="ea")
                nc.vector.scalar_tensor_tensor(
                    ea[:], e[:], NEG, e[:], op0=ALU.mult, op1=ALU.max
                )
                ex = wk.tile([P, G, K], F16, tag="ex")
                nc.scalar.activation(ex[:], ea[:], AF.Exp)

                dn = wk.tile([P, G], F16, tag="dn")
                nc.vector.tensor_reduce(dn[:], ex[:], axis=AX.X, op=ALU.add)
                nc.vector.tensor_scalar_add(dn[:], dn[:], 1e-4)
                inv = wk.tile([P, G], F16, tag="inv")
                nc.vector.reciprocal(inv[:], dn[:])

                msg = wk.tile([P, G, D2, K], F16, tag="msg")
                nc.vector.tensor_tensor(
                    msg[:],
                    evt[:, :, 0 : 16 * K].rearrange("p g (d k) -> p g d k", k=K),
                    _mid0(ex[:], 2, D2),
                    op=ALU.mult,
                )
                oacc = wk.tile([P, G, D2, 1], F16, tag="oacc")
                _tree_sum_k(nc, lambda a, b: msg[:, :, :, a:b], oacc[:], K)
                o = obuf[:, t0 : t0 + G, :]
                nc.vector.tensor_tensor(
                    o, oacc[:, :, :, 0], _tail0(inv[:], D2), op=ALU.mult
                )
                nc.vector.tensor_tensor(o, o, _bc(b2r[:], [P, G, D2]), op=ALU.add)

                # log_softmax part 1: subtract max, exp, accumulate denominator
                nm = wk.tile([P, G], F32, tag="nm")
                nc.vector.tensor_reduce(nm, o, axis=AX.X, op=ALU.max, negate=True)
                nc.vector.tensor_tensor(o, o, _tail0(nm[:], D2), op=ALU.add)
                exq = wk.tile([P, G, D2], F32, tag="exq")
                nc.scalar.activation(exq[:], o, AF.Exp)
                nc.vector.tensor_reduce(
                    ssbuf[:, t0 : t0 + G], exq[:], axis=AX.X, op=ALU.add
                )

            # log_softmax part 2: one Ln over all nodes, one subtract, one DMA
            lss = pro.tile([P, STEPS], F32)
            nc.scalar.activation(lss[:], ssbuf[:], AF.Ln)
            nc.vector.tensor_tensor(
                obuf[:], obuf[:], _tail0(lss[:], D2), op=ALU.subtract
            )
            nc.sync.dma_start(
                out[:].rearrange("(t p) f -> p t f", p=P), obuf[:]
            )
    nc.compile()
    return nc


# --------------------------------------------------------------------------
# Host orchestration
# --------------------------------------------------------------------------
def _make_groups(k_step, gmax, slot_budget):
    """Greedy: grow the group while tiles*K stays under slot_budget."""
    groups = []
    t0 = 0
    while t0 < STEPS:
        g = 1
        kb = max(int(k_step[t0]), 1)
        while (
            t0 + g < STEPS
            and g < gmax
            and (g + 1) * max(kb, int(k_step[t0 + g])) <= slot_budget
        ):
            kb = max(kb, int(k_step[t0 + g]))
            g += 1
        groups.append((t0, g, kb))
        t0 += g
    return groups


def _build_slots(groups, spos_node, deg, estart, src_by_dst):
    """slot -> src node id (N = pad) per core; layout per group is p-major:
    slot = base + p*(G*K) + g*K + k."""
    tot = sum(P * g * kb for _, g, kb in groups)
    slot = np.full((NC, tot), N, dtype=np.int64)
    arangeP = np.arange(P)
    for c in range(NC):
        base = 0
        for t0, g, kb in groups:
            for gi in range(g):
                T = (t0 + gi) * NC + c
                nodes = spos_node[T * P : (T + 1) * P]
                valid = nodes >= 0
                nv = nodes[valid]
                if nv.size == 0:
                    continue
                d = deg[nv]
                rowstart = base + arangeP[valid] * (g * kb) + gi * kb
                totd = int(d.sum())
                if totd == 0:
                    continue
                rep_row = np.repeat(rowstart, d)
                rep_cum = np.repeat(np.cumsum(d) - d, d)
                intra = np.arange(totd) - rep_cum
                rep_est = np.repeat(estart[nv], d)
                slot[c, rep_row + intra] = src_by_dst[rep_est + intra]
            base += P * g * kb
    return slot


def kernel(x, edge_index, W1, att_src1, att_dst1, b1, W2, att_src2, att_dst2, b2):
    x = np.asarray(x, dtype=np.float32)
    edge_index = np.asarray(edge_index)
    W1 = np.asarray(W1, dtype=np.float32)
    att_src1 = np.asarray(att_src1, dtype=np.float32)
    att_dst1 = np.asarray(att_dst1, dtype=np.float32)
    b1 = np.asarray(b1, dtype=np.float32)
    W2 = np.asarray(W2, dtype=np.float32)
    att_src2 = np.asarray(att_src2, dtype=np.float32).reshape(1, D2)
    att_dst2 = np.asarray(att_dst2, dtype=np.float32).reshape(1, D2)
    b2 = np.asarray(b2, dtype=np.float32)

    src = edge_index[0].astype(np.int64)
    dst = edge_index[1].astype(np.int64)

    # ---- schedule: degree-sorted tiles, round-robin dealt across cores ----
    deg = np.bincount(dst, minlength=N)
    order = np.argsort(deg, kind="stable")          # sorted-node space -> node id
    eo = np.argsort(dst, kind="stable")             # edges sorted by dst
    src_by_dst = src[eo]
    estart = np.zeros(N + 1, dtype=np.int64)
    estart[1:] = np.cumsum(deg)

    spos_node = np.full(TILES * P, -1, dtype=np.int64)
    spos_node[:N] = order
    sdeg = np.zeros(TILES * P, dtype=np.int64)
    sdeg[:N] = deg[order]
    tile_max = sdeg.reshape(TILES, P).max(axis=1)
    k_step = np.maximum(tile_max.reshape(STEPS, NC).max(axis=1), 1)  # [STEPS]
    k_step = ((k_step + 1) // 2) * 2       # even K: keeps fp16 rows 4B-aligned


    groups2 = _make_groups(k_step, 4, 96)
    groups3 = _make_groups(k_step, 8, 200)
    slots2 = _build_slots(groups2, spos_node, deg, estart, src_by_dst)
    slots3 = _build_slots(groups3, spos_node, deg, estart, src_by_dst)
    ad_rows = np.where(spos_node < 0, N, spos_node)  # [TILES*P] node per row
    # per-core view: row t*128+p of core c <-> sorted pos (t*NC+c)*128+p
    ad_rows = (
        ad_rows.reshape(STEPS, NC, P).transpose(1, 0, 2).reshape(NC, NPC)
    )

    # ---- K1: node tables ----
    xpad = np.zeros((NC * NPC, F_IN), dtype=np.float32)
    xpad[:N] = x
    nc1 = build_k1()
    in1 = [
        {
            # xh[p, t, c, j] = x[node t*128+j, feature c*128+p]
            "xh": np.ascontiguousarray(
                xpad[c * NPC : (c + 1) * NPC]
                .T.reshape(2, P, STEPS, P)
                .transpose(1, 2, 0, 3)
            ),
            "w1": W1,
            "as1": att_src1,
            "ad1": att_dst1,
        }
        for c in range(NC)
    ]
    r1 = _run(nc1, in1, "k1")
    xq1 = np.empty((NC * NPC + 1, 80), dtype=np.float16)
    for c in range(NC):
        xq1[c * NPC : (c + 1) * NPC] = r1[c]["xq1T"].T
    xq1[-1] = 0.0
    xq1[-1, 64:72] = PADS                           # pad row: s1 = -30000

    # ---- K2: layer 1 ----
    nc2 = build_k2(groups2, k_step)
    pad2 = np.where(slots2 >= N, NC * NPC, slots2)

    def _soa1(c):
        """One merged stream, per (group, p, g) blocks, k innermost:
        [s1 (8,K) | xp1 (8,8,K)]."""
        rows = xq1[pad2[c], 0:72]
        adT = xq1[np.where(ad_rows[c] >= N, NC * NPC, ad_rows[c]), 72:80]
        outc = np.empty(rows.shape[0] * 80, dtype=np.float16)
        bs = 0
        for t0, g, kb in groups2:
            n = P * g * kb
            arr = rows[bs : bs + n].reshape(P, g, kb, 72)
            s = arr[..., 64:72].transpose(0, 1, 3, 2).reshape(P, g * 8 * kb)
            adb = np.broadcast_to(
                adT[t0 * P : (t0 + g) * P]
                .reshape(g, P, 8)
                .transpose(1, 0, 2)[:, :, :, None],
                (P, g, 8, kb),
            ).reshape(P, g * 8 * kb)
            xp = (
                arr[..., 0:64]
                .reshape(P, g, kb, 8, 8)
                .transpose(0, 1, 3, 4, 2)
                .reshape(P, g * 64 * kb)
            )
            outc[bs * 80 : (bs + n) * 80] = np.concatenate(
                [s, adb, xp], axis=1
            ).ravel()
            bs += n
        return outc

    in2 = [
        {
            "ev1": _soa1(c),
            "w2": W2,
            "as2": att_src2,
            "ad2": att_dst2,
            "b1": b1,
        }
        for c in range(NC)
    ]
    r2 = _run(nc2, in2, "k2")

    # reassemble layer-2 node table in original-node space
    t2 = np.zeros((N + 1, 18), dtype=np.float16)
    t2[N, 16] = PADS                                # pad row: s2 = -30000
    for c in range(NC):
        cols = r2[c]["t2T"]                         # [18, NPC] fp16
        rows = cols.T.reshape(STEPS, P, 18)
        for t in range(STEPS):
            T = t * NC + c
            nodes = spos_node[T * P : (T + 1) * P]
            valid = nodes >= 0
            t2[nodes[valid]] = rows[t][valid]

    # ---- K3: layer 2 ----
    nc3 = build_k3(groups3)
    pad3 = np.where(slots3 >= N, N, slots3)

    def _soa2(c):
        """Per (group, p, g) blocks: [xp2 (16,K) | s2 (K) | ad2 (K)]."""
        rows = t2[pad3[c]]
        ad2T = t2[np.where(ad_rows[c] >= N, N, ad_rows[c]), 17]
        out = np.empty(rows.shape[0] * 18, dtype=np.float16)
        bs = 0
        bf = 0
        for t0, g, kb in groups3:
            n = P * g * kb
            arr = rows[bs : bs + n].reshape(P, g, kb, 18)
            xp = arr[..., 0:16].transpose(0, 1, 3, 2).reshape(P, g, 16 * kb)
            s = arr[..., 16].reshape(P, g, kb)
            adb = np.broadcast_to(
                ad2T[t0 * P : (t0 + g) * P].reshape(g, P).T[:, :, None],
                (P, g, kb),
            )
            out[bf : bf + n * 18] = np.concatenate([xp, s, adb], axis=2).ravel()
            bs += n
            bf += n * 18
        return out

    in3 = [
        {
            "ev2": _soa2(c),
            "b2": b2,
        }
        for c in range(NC)
    ]
    r3 = _run(nc3, in3, "k3")

    outp = np.zeros((N, D2), dtype=np.float32)
    for c in range(NC):
        o = r3[c]["o3"].reshape(STEPS, P, D2)
        for t in range(STEPS):
            T = t * NC + c
            nodes = spos_node[T * P : (T + 1) * P]
            valid = nodes >= 0
            outp[nodes[valid]] = o[t][valid]
    return outp
